# revision 5
# baseline (speedup 1.0000x reference)
"""3-layer GCN (DGL GraphConv norm='both' + ELU) on 8 TRN2 NeuronCores.

Strategy (node/data parallel, per sharding hint):
- Nodes sharded 12500/core; edges partitioned by dst core and sorted by dst,
  grouped into 128-dst-node tiles, padded to a cross-core-uniform chunk count
  per tile (SPMD requires one program for all cores).
- Per layer: h (bf16, node-major [N,64]) is AllGathered to every core; each
  core gathers h[src] per 128-edge chunk with indirect DMA, then segment-sums
  via a one-hot matmul: agg_tile += S^T @ msg, with S[e,d] = (dst_local==d)
  built on VectorE by comparing dst_local against an iota row.
- In-degree counts come free from an extra matmul against a ones column in
  layer 0; out-degrees from an identical counting pass over src-sorted edges.
- Tile epilogue: scale by in_norm, transpose (PE), z = agg @ W + b, ELU as
  max(z, exp(min(z,0))-1), scale by out_norm into the next layer's h.
"""
import math
import numpy as np
import ml_dtypes

N_NODES = 100000
N_EDGES = 1200000
D = 64
NCORES = 8
P = 128


# ---------------------------------------------------------------- host side
def _edge_chunks(key_nodes, other_nodes, n, ncores):
    """Partition edges by key_nodes's core, sort by key, tile by 128 key-locals,
    pad each (core,tile) run to a cross-core-uniform chunk count.

    Returns (T, kloc, oidx): T[t] = chunks for tile t (shared by all cores);
    kloc [ncores,128,C] bf16 key-local-in-tile (pad -1); oidx [ncores,128,C]
    int32 = other endpoint (pad 0)."""
    nsh = n // ncores
    ntiles = math.ceil(nsh / P)
    core = key_nodes // nsh
    tile = (key_nodes % nsh) // P
    flat = core * ntiles + tile
    counts = np.bincount(flat, minlength=ncores * ntiles).reshape(ncores, ntiles)
    T = np.maximum(1, -(-counts.max(axis=0) // P)).astype(np.int64)
    C = int(T.sum())
    starts = np.zeros(ntiles, np.int64)
    starts[1:] = np.cumsum(T)[:-1]

    order = np.argsort(flat, kind="stable")
    sflat = flat[order]
    run_start = np.searchsorted(sflat, np.arange(ncores * ntiles))
    pos = np.arange(key_nodes.shape[0]) - run_start[sflat]
    chunk = pos // P
    part = pos % P
    col = starts[tile[order]] + chunk

    kloc = np.full((ncores, P, C), -1.0, ml_dtypes.bfloat16)
    oidx = np.zeros((ncores, P, C), np.int32)
    kloc[core[order], part, col] = ((key_nodes[order] % nsh) % P).astype(
        ml_dtypes.bfloat16)
    oidx[core[order], part, col] = other_nodes[order].astype(np.int32)
    return T, kloc, oidx


def _preprocess(src, dst, n, ncores):
    T, dloc, gsrc = _edge_chunks(dst, src, n, ncores)       # main pass (by dst)
    T2, sloc, _ = _edge_chunks(src, dst, n, ncores)         # out-degree pass
    return T, dloc, gsrc, T2, sloc


# -------------------------------------------------------------- device side
def _build(n, ncores, T, T2):
    import concourse.bass as bass
    import concourse.bacc as bacc
    import concourse.mybir as mybir
    import concourse.tile as tile
    from concourse.masks import make_identity

    f32 = mybir.dt.float32
    bf16 = mybir.dt.bfloat16
    i32 = mybir.dt.int32
    AX = bass.IndirectOffsetOnAxis

    nsh = n // ncores
    ntiles = math.ceil(nsh / P)
    nrows = [min(P, nsh - t * P) for t in range(ntiles)]
    C = int(T.sum())
    C2 = int(T2.sum())
    starts = np.zeros(ntiles, np.int64)
    starts[1:] = np.cumsum(T)[:-1]
    starts2 = np.zeros(ntiles, np.int64)
    starts2[1:] = np.cumsum(T2)[:-1]
    full_t = 12 * (ntiles // 13) if ntiles >= 13 else ntiles  # tiles at 128 rows

    nc = bacc.Bacc("TRN2", target_bir_lowering=False, debug=False,
                   num_devices=ncores)
    feat = nc.declare_dram_parameter("feat", [nsh, D], f32, isOutput=False)
    Wcat = nc.declare_dram_parameter("Wcat", [3 * D, D], f32, isOutput=False)
    Bcat = nc.declare_dram_parameter("Bcat", [3 * P, D], f32, isOutput=False)
    gsrc = nc.declare_dram_parameter("gsrc", [P, C], i32, isOutput=False)
    dloc = nc.declare_dram_parameter("dloc", [P, C], bf16, isOutput=False)
    sloc = nc.declare_dram_parameter("sloc", [P, C2], bf16, isOutput=False)
    xout = nc.declare_dram_parameter("xout", [nsh, D], f32, isOutput=True)

    with tile.TileContext(nc) as tc:
        with (
            tc.tile_pool(name="dram", bufs=1, space="DRAM") as dram,
            tc.tile_pool(name="const", bufs=1) as cp,
            tc.tile_pool(name="msgs", bufs=12) as mp,
            tc.tile_pool(name="sels", bufs=6) as sp,
            tc.tile_pool(name="work", bufs=3) as wp,
            tc.tile_pool(name="psA", bufs=2, space="PSUM") as psA,
            tc.tile_pool(name="psB", bufs=2, space="PSUM") as psB,
            tc.tile_pool(name="psC", bufs=2, space="PSUM") as psC,
            tc.tile_pool(name="psN", bufs=2, space="PSUM") as psN,
        ):
            hsh = dram.tile([nsh, D], bf16)
            hfulls = [dram.tile([n, D], bf16, addr_space="Shared",
                                name=f"hfull{i}") for i in range(3)]

            # constants
            iota_i = cp.tile([P, P], i32)
            nc.gpsimd.iota(iota_i[:], pattern=[[1, P]], base=0,
                           channel_multiplier=0)
            iota_bf = cp.tile([P, P], bf16)
            nc.vector.tensor_copy(iota_bf[:], iota_i[:])
            ident = cp.tile([P, P], f32)
            make_identity(nc, ident[:])
            ones_bf = cp.tile([P, 1], bf16)
            nc.vector.memset(ones_bf[:], 1.0)

            W_sb = cp.tile([D, 3 * D], f32)
            nc.sync.dma_start(out=W_sb[:].rearrange("k (l d) -> k l d", l=3),
                              in_=Wcat[:].rearrange("(l k) d -> k l d", l=3))
            B_sb = cp.tile([P, 3 * D], f32)
            nc.sync.dma_start(out=B_sb[:].rearrange("p (l d) -> p l d", l=3),
                              in_=Bcat[:].rearrange("(l p) d -> p l d", l=3))
            gsrc_sb = cp.tile([P, C], i32)
            nc.sync.dma_start(out=gsrc_sb[:], in_=gsrc[:])
            dloc_sb = cp.tile([P, C], bf16)
            nc.sync.dma_start(out=dloc_sb[:], in_=dloc[:])
            sloc_sb = cp.tile([P, C2], bf16)
            nc.sync.dma_start(out=sloc_sb[:], in_=sloc[:])

            onorm = cp.tile([P, ntiles], f32)
            inorm = cp.tile([P, ntiles], f32)
            feat_acc = cp.tile([P, ntiles * D], f32)
            h_acc = cp.tile([P, ntiles * D], bf16)
            x_acc = cp.tile([P, ntiles * D], f32)

            def load_node_major(sb_acc, dram_t, dt):
                body = (ntiles - 1) * P
                nc.sync.dma_start(
                    out=sb_acc[:].rearrange("p (t d) -> p t d", d=D)[:, :ntiles - 1],
                    in_=dram_t[:body].rearrange("(t p) d -> p t d", p=P))
                tail = nsh - body
                nc.sync.dma_start(
                    out=sb_acc[:tail, (ntiles - 1) * D:],
                    in_=dram_t[body:])

            def store_node_major(dram_t, sb_acc):
                body = (ntiles - 1) * P
                nc.sync.dma_start(
                    out=dram_t[:body].rearrange("(t p) d -> p t d", p=P),
                    in_=sb_acc[:].rearrange("p (t d) -> p t d", d=D)[:, :ntiles - 1])
                tail = nsh - body
                nc.sync.dma_start(
                    out=dram_t[body:],
                    in_=sb_acc[:tail, (ntiles - 1) * D:])

            # ---- out-degree pass (src-sorted local edges, no gather) ----
            for t in range(ntiles):
                cnt = psN.tile([P, 1], f32, tag="cnt")
                for k in range(int(T2[t])):
                    col = int(starts2[t]) + k
                    S2 = sp.tile([P, P], bf16, tag="S")
                    nc.vector.tensor_tensor(
                        out=S2[:], in0=sloc_sb[:, col:col + 1].to_broadcast([P, P]),
                        in1=iota_bf[:], op=mybir.AluOpType.is_equal)
                    nc.tensor.matmul(cnt[:], lhsT=S2[:], rhs=ones_bf[:],
                                     start=(k == 0), stop=(k == int(T2[t]) - 1))
                nc.vector.tensor_scalar(out=onorm[:, t:t + 1], in0=cnt[:],
                                        scalar1=1.0, scalar2=None,
                                        op0=mybir.AluOpType.max)
            nc.scalar.activation(onorm[:], onorm[:],
                                 mybir.ActivationFunctionType.Sqrt)
            nc.vector.reciprocal(onorm[:], onorm[:])

            # ---- h0 = feat * out_norm ----
            load_node_major(feat_acc, feat, f32)
            for t in range(ntiles):
                nc.vector.tensor_scalar(
                    out=h_acc[:, t * D:(t + 1) * D],
                    in0=feat_acc[:, t * D:(t + 1) * D],
                    scalar1=onorm[:, t:t + 1], scalar2=None,
                    op0=mybir.AluOpType.mult)
            store_node_major(hsh, h_acc)

            # ---- layers ----
            for layer in range(3):
                hfull = hfulls[layer]
                nc.gpsimd.collective_compute(
                    "AllGather", mybir.AluOpType.bypass,
                    replica_groups=[list(range(ncores))],
                    ins=[hsh[:]], outs=[hfull[:]])
                Wl = W_sb[:, layer * D:(layer + 1) * D]
                Bl = B_sb[:, layer * D:(layer + 1) * D]
                for t in range(ntiles):
                    agg = psA.tile([P, D], f32, tag="agg")
                    cnt = psN.tile([P, 1], f32, tag="cnt")
                    for k in range(int(T[t])):
                        col = int(starts[t]) + k
                        msg = mp.tile([P, D], bf16, tag="msg")
                        nc.gpsimd.indirect_dma_start(
                            out=msg[:], out_offset=None, in_=hfull[:],
                            in_offset=AX(ap=gsrc_sb[:, col:col + 1], axis=0))
                        S = sp.tile([P, P], bf16, tag="S")
                        nc.vector.tensor_tensor(
                            out=S[:],
                            in0=dloc_sb[:, col:col + 1].to_broadcast([P, P]),
                            in1=iota_bf[:], op=mybir.AluOpType.is_equal)
                        nc.tensor.matmul(agg[:], lhsT=S[:], rhs=msg[:],
                                         start=(k == 0), stop=(k == int(T[t]) - 1))
                        if layer == 0:
                            nc.tensor.matmul(cnt[:], lhsT=S[:], rhs=ones_bf[:],
                                             start=(k == 0),
                                             stop=(k == int(T[t]) - 1))
                    if layer == 0:
                        nc.vector.tensor_scalar(out=inorm[:, t:t + 1], in0=cnt[:],
                                                scalar1=1.0, scalar2=None,
                                                op0=mybir.AluOpType.max)
                        nc.scalar.activation(inorm[:, t:t + 1], inorm[:, t:t + 1],
                                             mybir.ActivationFunctionType.Sqrt)
                        nc.vector.reciprocal(inorm[:, t:t + 1], inorm[:, t:t + 1])
                    # epilogue: z = (agg*inorm) @ W + b ; x = elu(z)
                    aggs = wp.tile([P, D], f32, tag="aggs")
                    nc.vector.tensor_scalar(out=aggs[:], in0=agg[:],
                                            scalar1=inorm[:, t:t + 1], scalar2=None,
                                            op0=mybir.AluOpType.mult)
                    psT = psB.tile([D, P], f32, tag="psT")
                    nc.tensor.transpose(psT[:], aggs[:], ident[:])
                    aggT = wp.tile([D, P], f32, tag="aggT")
                    nc.vector.tensor_copy(aggT[:], psT[:])
                    z = psC.tile([P, D], f32, tag="z")
                    nc.tensor.matmul(z[:], lhsT=aggT[:], rhs=Wl, start=True,
                                     stop=True)
                    zb = wp.tile([P, D], f32, tag="zb")
                    nc.vector.tensor_tensor(out=zb[:], in0=z[:], in1=Bl,
                                            op=mybir.AluOpType.add)
                    m0 = wp.tile([P, D], f32, tag="m0")
                    nc.vector.tensor_scalar(out=m0[:], in0=zb[:], scalar1=0.0,
                                            scalar2=None, op0=mybir.AluOpType.min)
                    ex = wp.tile([P, D], f32, tag="ex")
                    nc.scalar.activation(ex[:], m0[:],
                                         mybir.ActivationFunctionType.Exp)
                    e1 = wp.tile([P, D], f32, tag="e1")
                    nc.vector.tensor_scalar(out=e1[:], in0=ex[:], scalar1=-1.0,
                                            scalar2=None, op0=mybir.AluOpType.add)
                    if layer < 2:
                        xe = wp.tile([P, D], f32, tag="xe")
                        nc.vector.tensor_tensor(out=xe[:], in0=zb[:], in1=e1[:],
                                                op=mybir.AluOpType.max)
                        nc.vector.tensor_scalar(
                            out=h_acc[:, t * D:(t + 1) * D], in0=xe[:],
                            scalar1=onorm[:, t:t + 1], scalar2=None,
                            op0=mybir.AluOpType.mult)
                    else:
                        nc.vector.tensor_tensor(
                            out=x_acc[:, t * D:(t + 1) * D], in0=zb[:], in1=e1[:],
                            op=mybir.AluOpType.max)
                if layer < 2:
                    store_node_major(hsh, h_acc)
            store_node_major(xout, x_acc)

    nc.compile()
    return nc


# ------------------------------------------------------------------- driver
def kernel(feat, W0, b0, W1, b1, W2, b2, src, dst):
    from concourse.bass_utils import run_bass_kernel_spmd

    n, ncores = N_NODES, NCORES
    nsh = n // ncores
    feat = np.asarray(feat, np.float32)
    src = np.asarray(src).astype(np.int64)
    dst = np.asarray(dst).astype(np.int64)

    T, dloc, gsrc, T2, sloc = _preprocess(src, dst, n, ncores)
    nc = _build(n, ncores, T, T2)

    Wcat = np.concatenate([np.asarray(W, np.float32) for W in (W0, W1, W2)], 0)
    Bcat = np.concatenate(
        [np.broadcast_to(np.asarray(b, np.float32), (P, D)) for b in (b0, b1, b2)], 0)
    in_maps = []
    for c in range(ncores):
        in_maps.append({
            "feat": np.ascontiguousarray(feat[c * nsh:(c + 1) * nsh]),
            "Wcat": Wcat, "Bcat": np.ascontiguousarray(Bcat),
            "gsrc": np.ascontiguousarray(gsrc[c]),
            "dloc": np.ascontiguousarray(dloc[c]),
            "sloc": np.ascontiguousarray(sloc[c]),
        })
    res = run_bass_kernel_spmd(nc, in_maps, list(range(ncores)), trace=False)
    out = np.concatenate([res.results[c]["xout"] for c in range(ncores)], axis=0)
    return np.ascontiguousarray(out, dtype=np.float32)


# revision 7
# speedup vs baseline: 1.0798x; 1.0798x over previous
"""3-layer GCN (DGL GraphConv norm='both' + ELU) on 8 TRN2 NeuronCores.

Strategy (node/data parallel, per sharding hint):
- Nodes are relabeled so every 128-node dst tile carries a near-equal number
  of incoming edges (greedy balance on in-degree), then sharded 12500/core.
- Edges are partitioned by dst core, sorted by dst, grouped into 128-dst-node
  tiles, padded to a cross-core-uniform chunk count per tile (SPMD: one
  program for all 8 cores).
- Per layer: h (bf16, node-major [N,64]) is AllGathered; each core gathers
  h[src] per 128-edge chunk with indirect DMA (the critical resource: Q7
  descriptor generation at ~9ns/row), then segment-sums via one-hot matmul:
  agg_tile += S^T @ msg with S[e,d] = (dst_local==d), built on VectorE by
  comparing dst_local against an iota row (batched 8 chunks per op).
- In-degree counts come free from an extra matmul against a ones column in
  layer 0; out-degrees from an identical counting pass over src-sorted edges.
- Tile epilogue: scale by in_norm, transpose (PE), z = agg @ W + b, ELU as
  max(z, exp(min(z,0))-1), scale by out_norm into the next layer's h.
"""
import math
import numpy as np
import ml_dtypes

N_NODES = 100000
N_EDGES = 1200000
D = 64
NCORES = 8
P = 128
SGRP = 8  # chunks per batched one-hot build


# ---------------------------------------------------------------- host side
def _balance_relabel(dst, n, ncores):
    """Relabel nodes so per-(core,tile) in-degree sums are near-uniform.
    Returns newlabel[old_node]."""
    nsh = n // ncores
    ntiles = math.ceil(nsh / P)
    nbins = ncores * ntiles
    lastcap = nsh - (ntiles - 1) * P
    cap = np.full(nbins, P, np.int64)
    cap[np.arange(nbins) % ntiles == ntiles - 1] = lastcap

    deg = np.bincount(dst, minlength=n).astype(np.int64)
    order = np.argsort(-deg, kind="stable")
    load = np.zeros(nbins, np.int64)
    fill = np.zeros(nbins, np.int64)
    binof = np.empty(n, np.int64)
    slot = np.empty(n, np.int64)
    pos = 0
    while pos < n:
        active = np.where(fill < cap)[0]
        k = min(active.size, n - pos)
        bins = active[np.argsort(load[active], kind="stable")[:k]]
        nodes = order[pos:pos + k]
        binof[nodes] = bins
        slot[nodes] = fill[bins]
        load[bins] += deg[nodes]
        fill[bins] += 1
        pos += k
    core, tile = binof // ntiles, binof % ntiles
    return core * nsh + tile * P + slot


def _edge_chunks(key_nodes, other_nodes, n, ncores):
    """Partition edges by key core, sort by key, tile by 128 key-locals, pad
    each (core,tile) run to a cross-core-uniform chunk count. Returns
    (T, kloc, oidx): T[t] chunks per tile; kloc [ncores,128,C] bf16
    key-local (pad -1); oidx [ncores,128,C] int32 other endpoint (pad 0)."""
    nsh = n // ncores
    ntiles = math.ceil(nsh / P)
    core = key_nodes // nsh
    tile = (key_nodes % nsh) // P
    flat = core * ntiles + tile
    counts = np.bincount(flat, minlength=ncores * ntiles).reshape(ncores, ntiles)
    T = np.maximum(1, -(-counts.max(axis=0) // P)).astype(np.int64)
    C = int(T.sum())
    starts = np.zeros(ntiles, np.int64)
    starts[1:] = np.cumsum(T)[:-1]

    order = np.argsort(flat, kind="stable")
    sflat = flat[order]
    run_start = np.searchsorted(sflat, np.arange(ncores * ntiles))
    pos = np.arange(key_nodes.shape[0]) - run_start[sflat]
    col = starts[tile[order]] + pos // P

    kloc = np.full((ncores, P, C), -1.0, ml_dtypes.bfloat16)
    oidx = np.zeros((ncores, P, C), np.int32)
    kloc[core[order], pos % P, col] = ((key_nodes[order] % nsh) % P).astype(
        ml_dtypes.bfloat16)
    oidx[core[order], pos % P, col] = other_nodes[order].astype(np.int32)
    return T, kloc, oidx


# -------------------------------------------------------------- device side
def _build(n, ncores, T, T2):
    import concourse.bass as bass
    import concourse.bacc as bacc
    import concourse.mybir as mybir
    import concourse.tile as tile
    from concourse.masks import make_identity

    f32 = mybir.dt.float32
    bf16 = mybir.dt.bfloat16
    i32 = mybir.dt.int32
    AX = bass.IndirectOffsetOnAxis
    EQ = mybir.AluOpType.is_equal

    nsh = n // ncores
    ntiles = math.ceil(nsh / P)
    C = int(T.sum())
    C2 = int(T2.sum())
    starts = np.zeros(ntiles, np.int64)
    starts[1:] = np.cumsum(T)[:-1]
    starts2 = np.zeros(ntiles, np.int64)
    starts2[1:] = np.cumsum(T2)[:-1]

    nc = bacc.Bacc("TRN2", target_bir_lowering=False, debug=False,
                   num_devices=ncores)
    feat = nc.declare_dram_parameter("feat", [nsh, D], f32, isOutput=False)
    Wcat = nc.declare_dram_parameter("Wcat", [3 * D, D], f32, isOutput=False)
    Bcat = nc.declare_dram_parameter("Bcat", [3 * P, D], f32, isOutput=False)
    gsrc = nc.declare_dram_parameter("gsrc", [P, C], i32, isOutput=False)
    dloc = nc.declare_dram_parameter("dloc", [P, C], bf16, isOutput=False)
    sloc = nc.declare_dram_parameter("sloc", [P, C2], bf16, isOutput=False)
    xout = nc.declare_dram_parameter("xout", [nsh, D], f32, isOutput=True)

    with tile.TileContext(nc) as tc:
        with (
            tc.tile_pool(name="dram", bufs=1, space="DRAM") as dram,
            tc.tile_pool(name="const", bufs=1) as cp,
            tc.tile_pool(name="msgs", bufs=16) as mp,
            tc.tile_pool(name="sels", bufs=4) as sp,
            tc.tile_pool(name="work", bufs=3) as wp,
            tc.tile_pool(name="psA", bufs=2, space="PSUM") as psA,
            tc.tile_pool(name="psB", bufs=2, space="PSUM") as psB,
            tc.tile_pool(name="psC", bufs=2, space="PSUM") as psC,
            tc.tile_pool(name="psN", bufs=2, space="PSUM") as psN,
        ):
            hsh = dram.tile([nsh, D], bf16)
            hfulls = [dram.tile([n, D], bf16, addr_space="Shared",
                                name=f"hfull{i}") for i in range(3)]

            # constants
            iota_i = cp.tile([P, P], i32)
            nc.gpsimd.iota(iota_i[:], pattern=[[1, P]], base=0,
                           channel_multiplier=0)
            iota_bf = cp.tile([P, P], bf16)
            nc.vector.tensor_copy(iota_bf[:], iota_i[:])
            ident = cp.tile([P, P], f32)
            make_identity(nc, ident[:])
            ones_bf = cp.tile([P, 1], bf16)
            nc.vector.memset(ones_bf[:], 1.0)

            W_sb = cp.tile([D, 3 * D], f32)
            nc.sync.dma_start(out=W_sb[:].rearrange("k (l d) -> k l d", l=3),
                              in_=Wcat[:].rearrange("(l k) d -> k l d", l=3))
            B_sb = cp.tile([P, 3 * D], f32)
            nc.sync.dma_start(out=B_sb[:].rearrange("p (l d) -> p l d", l=3),
                              in_=Bcat[:].rearrange("(l p) d -> p l d", l=3))
            gsrc_sb = cp.tile([P, C], i32)
            nc.sync.dma_start(out=gsrc_sb[:], in_=gsrc[:])
            dloc_sb = cp.tile([P, C], bf16)
            nc.sync.dma_start(out=dloc_sb[:], in_=dloc[:])
            sloc_sb = cp.tile([P, C2], bf16)
            nc.sync.dma_start(out=sloc_sb[:], in_=sloc[:])

            onorm = cp.tile([P, ntiles], f32)
            inorm = cp.tile([P, ntiles], f32)
            feat_acc = cp.tile([P, ntiles * D], f32)
            h_acc = cp.tile([P, ntiles * D], bf16)
            x_acc = cp.tile([P, ntiles * D], f32)

            def build_sel_group(src_sb, g, cmax):
                """One is_equal over up to SGRP chunks:
                S8[:, j*P + d] = (src_sb[:, g*SGRP+j] == d)."""
                w = min(SGRP, cmax - g * SGRP)
                S8 = sp.tile([P, SGRP * P], bf16, tag="S8", name="S8")
                lo = g * SGRP
                in0 = src_sb[:, lo:lo + w].to_broadcast([P, w, P])
                it = iota_bf[:]
                in1 = bass.AP(it.tensor, it.offset,
                              [list(it.ap[0]), [0, w], list(it.ap[1])])
                o = S8[:].rearrange("p (c d) -> p c d", d=P)[:, :w]
                nc.vector.tensor_tensor(out=o, in0=in0, in1=in1, op=EQ)
                return S8

            def load_node_major(sb_acc, dram_t):
                body = (ntiles - 1) * P
                nc.sync.dma_start(
                    out=sb_acc[:].rearrange("p (t d) -> p t d", d=D)[:, :ntiles - 1],
                    in_=dram_t[:body].rearrange("(t p) d -> p t d", p=P))
                nc.sync.dma_start(
                    out=sb_acc[:nsh - body, (ntiles - 1) * D:],
                    in_=dram_t[body:])

            def store_node_major(dram_t, sb_acc):
                body = (ntiles - 1) * P
                nc.sync.dma_start(
                    out=dram_t[:body].rearrange("(t p) d -> p t d", p=P),
                    in_=sb_acc[:].rearrange("p (t d) -> p t d", d=D)[:, :ntiles - 1])
                nc.sync.dma_start(
                    out=dram_t[body:],
                    in_=sb_acc[:nsh - body, (ntiles - 1) * D:])

            # ---- out-degree pass (src-sorted local edges, no gather) ----
            S8 = None
            for t in range(ntiles):
                cnt = psN.tile([P, 1], f32, tag="cnt")
                for k in range(int(T2[t])):
                    col = int(starts2[t]) + k
                    if col % SGRP == 0:
                        S8 = build_sel_group(sloc_sb, col // SGRP, C2)
                    Sv = S8[:, (col % SGRP) * P:(col % SGRP + 1) * P]
                    nc.tensor.matmul(cnt[:], lhsT=Sv, rhs=ones_bf[:],
                                     start=(k == 0), stop=(k == int(T2[t]) - 1))
                nc.vector.tensor_scalar(out=onorm[:, t:t + 1], in0=cnt[:],
                                        scalar1=1.0, scalar2=None,
                                        op0=mybir.AluOpType.max)
            nc.scalar.activation(onorm[:], onorm[:],
                                 mybir.ActivationFunctionType.Sqrt)
            nc.vector.reciprocal(onorm[:], onorm[:])

            # ---- h0 = feat * out_norm ----
            load_node_major(feat_acc, feat)
            for t in range(ntiles):
                nc.vector.tensor_scalar(
                    out=h_acc[:, t * D:(t + 1) * D],
                    in0=feat_acc[:, t * D:(t + 1) * D],
                    scalar1=onorm[:, t:t + 1], scalar2=None,
                    op0=mybir.AluOpType.mult)
            store_node_major(hsh, h_acc)

            # ---- layers ----
            for layer in range(3):
                hfull = hfulls[layer]
                nc.gpsimd.collective_compute(
                    "AllGather", mybir.AluOpType.bypass,
                    replica_groups=[list(range(ncores))],
                    ins=[hsh[:]], outs=[hfull[:]])
                Wl = W_sb[:, layer * D:(layer + 1) * D]
                Bl = B_sb[:, layer * D:(layer + 1) * D]
                S8 = None
                for t in range(ntiles):
                    agg = psA.tile([P, D], f32, tag="agg")
                    cnt = psN.tile([P, 1], f32, tag="cnt")
                    for k in range(int(T[t])):
                        col = int(starts[t]) + k
                        msg = mp.tile([P, D], bf16, tag="msg")
                        nc.gpsimd.indirect_dma_start(
                            out=msg[:], out_offset=None, in_=hfull[:],
                            in_offset=AX(ap=gsrc_sb[:, col:col + 1], axis=0))
                        if col % SGRP == 0:
                            S8 = build_sel_group(dloc_sb, col // SGRP, C)
                        Sv = S8[:, (col % SGRP) * P:(col % SGRP + 1) * P]
                        nc.tensor.matmul(agg[:], lhsT=Sv, rhs=msg[:],
                                         start=(k == 0), stop=(k == int(T[t]) - 1))
                        if layer == 0:
                            nc.tensor.matmul(cnt[:], lhsT=Sv, rhs=ones_bf[:],
                                             start=(k == 0),
                                             stop=(k == int(T[t]) - 1))
                    if layer == 0:
                        nc.vector.tensor_scalar(out=inorm[:, t:t + 1], in0=cnt[:],
                                                scalar1=1.0, scalar2=None,
                                                op0=mybir.AluOpType.max)
                        nc.scalar.activation(inorm[:, t:t + 1], inorm[:, t:t + 1],
                                             mybir.ActivationFunctionType.Sqrt)
                        nc.vector.reciprocal(inorm[:, t:t + 1], inorm[:, t:t + 1])
                    # epilogue: z = (agg*inorm) @ W + b ; x = elu(z)
                    aggs = wp.tile([P, D], f32, tag="aggs")
                    nc.vector.tensor_scalar(out=aggs[:], in0=agg[:],
                                            scalar1=inorm[:, t:t + 1], scalar2=None,
                                            op0=mybir.AluOpType.mult)
                    psT = psB.tile([D, P], f32, tag="psT")
                    nc.tensor.transpose(psT[:], aggs[:], ident[:])
                    aggT = wp.tile([D, P], f32, tag="aggT")
                    nc.vector.tensor_copy(aggT[:], psT[:])
                    z = psC.tile([P, D], f32, tag="z")
                    nc.tensor.matmul(z[:], lhsT=aggT[:], rhs=Wl, start=True,
                                     stop=True)
                    zb = wp.tile([P, D], f32, tag="zb")
                    nc.vector.tensor_tensor(out=zb[:], in0=z[:], in1=Bl,
                                            op=mybir.AluOpType.add)
                    m0 = wp.tile([P, D], f32, tag="m0")
                    nc.vector.tensor_scalar(out=m0[:], in0=zb[:], scalar1=0.0,
                                            scalar2=None, op0=mybir.AluOpType.min)
                    ex = wp.tile([P, D], f32, tag="ex")
                    nc.scalar.activation(ex[:], m0[:],
                                         mybir.ActivationFunctionType.Exp)
                    e1 = wp.tile([P, D], f32, tag="e1")
                    nc.vector.tensor_scalar(out=e1[:], in0=ex[:], scalar1=-1.0,
                                            scalar2=None, op0=mybir.AluOpType.add)
                    if layer < 2:
                        xe = wp.tile([P, D], f32, tag="xe")
                        nc.vector.tensor_tensor(out=xe[:], in0=zb[:], in1=e1[:],
                                                op=mybir.AluOpType.max)
                        nc.vector.tensor_scalar(
                            out=h_acc[:, t * D:(t + 1) * D], in0=xe[:],
                            scalar1=onorm[:, t:t + 1], scalar2=None,
                            op0=mybir.AluOpType.mult)
                    else:
                        nc.vector.tensor_tensor(
                            out=x_acc[:, t * D:(t + 1) * D], in0=zb[:], in1=e1[:],
                            op=mybir.AluOpType.max)
                if layer < 2:
                    store_node_major(hsh, h_acc)
            store_node_major(xout, x_acc)

    nc.compile()
    return nc


# ------------------------------------------------------------------- driver
def _run(feat, Ws, bs, src, dst, n, ncores, trace=False):
    from concourse.bass_utils import run_bass_kernel_spmd

    nsh = n // ncores
    feat = np.asarray(feat, np.float32)
    src = np.asarray(src).astype(np.int64)
    dst = np.asarray(dst).astype(np.int64)

    newlab = _balance_relabel(dst, n, ncores)
    old_of_new = np.empty(n, np.int64)
    old_of_new[newlab] = np.arange(n)
    src2, dst2 = newlab[src], newlab[dst]
    feat2 = feat[old_of_new]

    T, dloc, gsrc = _edge_chunks(dst2, src2, n, ncores)
    T2, sloc, _ = _edge_chunks(src2, dst2, n, ncores)
    nc = _build(n, ncores, T, T2)

    Wcat = np.concatenate([np.asarray(W, np.float32) for W in Ws], 0)
    Bcat = np.concatenate(
        [np.broadcast_to(np.asarray(b, np.float32), (P, D)) for b in bs], 0)
    in_maps = [{
        "feat": np.ascontiguousarray(feat2[c * nsh:(c + 1) * nsh]),
        "Wcat": Wcat, "Bcat": np.ascontiguousarray(Bcat),
        "gsrc": np.ascontiguousarray(gsrc[c]),
        "dloc": np.ascontiguousarray(dloc[c]),
        "sloc": np.ascontiguousarray(sloc[c]),
    } for c in range(ncores)]
    res = run_bass_kernel_spmd(nc, in_maps, list(range(ncores)), trace=trace)
    out2 = np.concatenate([res.results[c]["xout"] for c in range(ncores)], 0)
    out = out2[newlab]
    return np.ascontiguousarray(out, dtype=np.float32), res, (T, T2)


def kernel(feat, W0, b0, W1, b1, W2, b2, src, dst):
    out, _, _ = _run(feat, (W0, W1, W2), (b0, b1, b2), src, dst,
                     N_NODES, NCORES, trace=False)
    return out
